# revision 1
# baseline (speedup 1.0000x reference)
"""Trainium2 Bass kernel for nn_DMS_STGAT (dual-branch GAT attention softmaxes).

Strategy (per core, data-parallel over batch B=16 -> 2 per core):
  The reference only uses h = x @ W through two dots s1 = h@a[:F], s2 = h@a[F:],
  so  e[bt, n1, n2] = LRelu(s1[r1[m]] + s2[r2[m]])  with fixed index maps r1/r2
  (the "scrambled pairing").  s1/s2 are 128-dim dots against host-precomputed
  Wa = W@a vectors, gathered via tiny host-precomputed 0/1 matrices using
  stacked-K PE matmuls; the double softmax runs on-chip.  Spatial PE term
  exp(-||x_j - x_8||/1000) rides extra stacked rows with Qs = S1*Q1 + S2*Q2.
  Temporal positional constant qp rides a ones-row; exp-overflow safety comes
  from a post-LRelu per-group constant shift.

  Batch rows inside the kernel are ordered (ts, b, ck) / (js, b, jq) so that
  all on-chip corner-turn DMAs are contiguous block copies; per-ts output DMAs
  unscramble to the reference layout.
"""
import sys
import numpy as np

for _p in ("/opt/trn_rl_repo", "/root/.axon_site/_ro/trn_rl_repo"):
    if _p not in sys.path:
        sys.path.insert(0, _p)

from contextlib import ExitStack  # noqa: E402

import concourse.bass as bass  # noqa: E402
import concourse.tile as tile  # noqa: E402
from concourse import bacc, mybir  # noqa: E402

B, C, T, J, F = 16, 128, 25, 25, 256
N = 25            # N == T == J
NN = N * N        # 625
NCORES = 8
BL = B // NCORES  # 2 batches per core
FP = mybir.dt.float32
BF = mybir.dt.bfloat16
AF = mybir.ActivationFunctionType
ALU = mybir.AluOpType

KS = 89           # spatial stack: 0:25 s1, 25:50 s2, 50:64 zero, 64:89 ec
KT = 57           # temporal stack: 0:25 t1, 25 ones, 26:32 zero, 32:57 t2

# n2-split for softmax-chain pipelining
N2SPLITS = [(0, 13), (13, 25)]

# Pin ALL activation functions to one table set (exp/ln/square/copy live
# together in natural_log_exp_and_others) so only one ACT_TABLE_LOAD happens.
_orig_get_tables = bacc.get_activation_tables


def _pinned_tables(arch):
    tabs = dict(_orig_get_tables(arch))
    assert "natural_log_exp_and_others" in tabs
    return {k: (v if k == "natural_log_exp_and_others" else set())
            for k, v in tabs.items()}


bacc.get_activation_tables = _pinned_tables

# ---------------------------------------------------------------- host math --

def _pair_indices():
    r1 = np.zeros(NN, np.int64)
    r2 = np.zeros(NN, np.int64)
    for m in range(NN):
        k1, k2 = 2 * m, 2 * m + 1
        r1[m] = (k1 // N) if k1 < NN else ((k1 - NN) % N)
        r2[m] = (k2 // N) if k2 < NN else ((k2 - NN) % N)
    return r1, r2


def _sinusoid_pos():
    pos = np.arange(200)[:, None].astype(np.float64)
    hid = np.arange(C)[None, :]
    angle = pos / np.power(10000.0, 2.0 * (hid // 2) / C)
    tab = angle.copy()
    tab[:, 0::2] = np.sin(angle[:, 0::2])
    tab[:, 1::2] = np.cos(angle[:, 1::2])
    return tab[:T] * 1000.0  # [T, C] float64


_R1, _R2 = _pair_indices()


def _host_consts(W_s, a_s, W_t, a_t):
    """Precompute tiny derived params in float64. ~0.3 MFLOP."""
    W_s = W_s.astype(np.float64)
    a_s = a_s.astype(np.float64)
    W_t = W_t.astype(np.float64)
    a_t = a_t.astype(np.float64)
    wa_s1 = W_s @ a_s[:F, 0]
    wa_s2 = W_s @ a_s[F:, 0]
    wa_t1 = W_t @ a_t[:F, 0]
    wa_t2 = W_t @ a_t[F:, 0]
    S1, S2 = wa_s1.sum(), wa_s2.sum()

    Q1 = np.zeros((N, NN), np.float64)
    Q2 = np.zeros((N, NN), np.float64)
    Q1[_R1, np.arange(NN)] = 1.0
    Q2[_R2, np.arange(NN)] = 1.0
    qs = S1 * Q1 + S2 * Q2

    pos = _sinusoid_pos()
    p1 = pos @ wa_t1
    p2 = pos @ wa_t2
    qp = p1[_R1] + p2[_R2]
    qLR = np.where(qp > 0, qp, 0.2 * qp)
    cq = qLR.reshape(N, N).max(axis=0)
    csh = cq[np.arange(NN) % N][None, :]        # [1, 625]

    wa4 = np.stack([wa_s1, wa_s2, wa_t1, wa_t2], axis=1)  # [128, 4]
    # permute the m-axis to n2-major (m' = n2*25 + n1) so the softmax chain
    # and its n1-group reductions are contiguous on-chip
    mperm = (np.arange(NN) % N) * N + (np.arange(NN) // N)  # m' -> orig m
    qstk_s = np.zeros((KS, NN), np.float64)
    qstk_s[0:N] = Q1[:, mperm]
    qstk_s[N:2 * N] = Q2[:, mperm]
    qstk_s[64:64 + N] = qs[:, mperm]
    qstk_t = np.zeros((KT, NN), np.float64)
    qstk_t[0:N] = Q1[:, mperm]
    qstk_t[N] = qp[mperm]
    qstk_t[32:32 + N] = Q2[:, mperm]
    csh = cq[np.arange(NN) // N][None, :]       # n2-major
    return (wa4.astype(np.float32), qstk_s.astype(np.float32),
            qstk_t.astype(np.float32), csh.astype(np.float32))


# ------------------------------------------------------------- bass program --

def _build_program():
    nc = bacc.Bacc("TRN2", target_bir_lowering=False, debug=False)

    src_d = nc.dram_tensor("src_l", [BL, C, T, J], FP, kind="ExternalInput").ap()
    wa4_d = nc.dram_tensor("wa4", [C, 4], FP, kind="ExternalInput").ap()
    qss_d = nc.dram_tensor("qstk_s", [KS, NN], FP, kind="ExternalInput").ap()
    qst_d = nc.dram_tensor("qstk_t", [KT, NN], FP, kind="ExternalInput").ap()
    csh_d = nc.dram_tensor("csh", [1, NN], FP, kind="ExternalInput").ap()
    outs_d = nc.dram_tensor("out_s", [BL, T, N, N], FP, kind="ExternalOutput").ap()
    outt_d = nc.dram_tensor("out_t", [BL, T, N, N], FP, kind="ExternalOutput").ap()

    with tile.TileContext(nc) as tc, ExitStack() as ctx:
        consts = ctx.enter_context(tc.tile_pool(name="consts", bufs=1))
        data = ctx.enter_context(tc.tile_pool(name="data", bufs=1))
        pp = ctx.enter_context(tc.tile_pool(name="pp", bufs=1, space="PSUM"))

        # --- tiny consts first, then the input batches on parallel rings ---
        wa4 = consts.tile([C, 4], FP)
        nc.sync.dma_start(wa4[:], wa4_d)
        X = data.tile([C, BL * NN], FP)
        for b, eng in ((0, nc.sync), (1, nc.scalar)):
            src_b = bass.AP(tensor=src_d.tensor, offset=src_d.offset + b * C * NN,
                            ap=[[NN, C], [1, NN]])
            eng.dma_start(X[:, b * NN:(b + 1) * NN], src_b)
        FX = X[:].ap[0][0]

        # --- ACT table warm-up ---
        dummy = consts.tile([1, 2], FP)
        nc.vector.memset(dummy[:], 0.0)
        nc.scalar.activation(dummy[:], dummy[:], AF.Exp)

        ones_bf = consts.tile([C, 1], BF)
        nc.vector.memset(ones_bf[:], 1.0)

        # --- X_jt [128, (b, j, t)] for the temporal pass (ACT strided copy) ---
        X_jt = data.tile([C, BL * NN], FP)
        for b in range(BL):
            xin = bass.AP(tensor=X.tensor, offset=X.offset + b * NN,
                          ap=[[FX, C], [1, N], [N, N]])   # (c, j, t)
            nc.scalar.copy(X_jt[:, b * NN:(b + 1) * NN], xin)

        # --- big consts on the ACT queue (issued after X_jt; needed at E) ---
        qst = consts.tile([KT, NN], FP)
        nc.scalar.dma_start(qst[:], qst_d)
        qss = consts.tile([KS, NN], FP)
        nc.scalar.dma_start(qss[:], qss_d)

        # --- D2 = (X - ref)^2 in bf16, per b ---
        D = data.tile([C, BL * NN], FP)
        D2 = data.tile([C, BL * NN], BF)
        FD = D[:].ap[0][0]
        for b in range(BL):
            in0 = bass.AP(tensor=X.tensor, offset=X.offset + b * NN,
                          ap=[[FX, C], [N, N], [1, N]])
            ref = bass.AP(tensor=X.tensor, offset=X.offset + b * NN + 8,
                          ap=[[FX, C], [N, N], [0, N]])
            dout = bass.AP(tensor=D.tensor, offset=D.offset + b * NN,
                           ap=[[FD, C], [N, N], [1, N]])
            nc.gpsimd.tensor_tensor(dout, in0, ref, op=ALU.subtract)
            eng = nc.vector if b == 0 else nc.gpsimd
            eng.tensor_tensor(D2[:, b * NN:(b + 1) * NN],
                              D[:, b * NN:(b + 1) * NN],
                              D[:, b * NN:(b + 1) * NN], op=ALU.mult)

        # --- PE dot passes (chunked stationary) ---
        psum_E = pp.tile([114, 1024], FP)  # first: keeps 512-chunks bank-aligned
        psum_sd = pp.tile([125, 30], FP)   # col (b*5+ck)*3 + {0:s1,1:s2,2:d2}
        psum_td = pp.tile([125, 20], FP)   # col (b*5+jq)*2 + {t1,t2}
        for b in range(BL):
            for ck in range(5):
                q = b * 5 + ck
                nc.tensor.matmul(psum_sd[:, q * 3:q * 3 + 2],
                                 X[:, q * 125:(q + 1) * 125], wa4[:, 0:2],
                                 start=True, stop=True)
            for ck in range(5):
                q = b * 5 + ck
                nc.tensor.matmul(psum_td[:, q * 2:q * 2 + 2],
                                 X_jt[:, q * 125:(q + 1) * 125], wa4[:, 2:4],
                                 start=True, stop=True)
            for ck in range(5):
                q = b * 5 + ck
                nc.tensor.matmul(psum_sd[:, q * 3 + 2:q * 3 + 3],
                                 D2[:, q * 125:(q + 1) * 125], ones_bf[:],
                                 start=True, stop=True)

        # --- PSUM -> SBUF with d-major column permute (lane-local) ---
        # TDp[p, d*10 + bjq] = psum_td[p, bjq*2 + d]
        TDp = data.tile([125, 20], FP)
        FTD = TDp[:].ap[0][0]
        td_out = bass.AP(tensor=TDp.tensor, offset=TDp.offset,
                         ap=[[FTD, 125], [1, 10], [10, 2]])      # (bjq, d)
        td_in = bass.AP(tensor=psum_td.tensor, offset=psum_td.offset,
                        ap=[[psum_td[:].ap[0][0], 125], [2, 10], [1, 2]])
        nc.vector.tensor_copy(td_out, td_in)
        # SDp[p, d*10 + bck] = psum_sd[p, bck*3 + d]; split so the s1/s2
        # planes copy out as soon as the spatial MMs finish (d2 comes later)
        SDp = data.tile([125, 30], FP)
        FSD = SDp[:].ap[0][0]
        FPS = psum_sd[:].ap[0][0]
        sd_outA = bass.AP(tensor=SDp.tensor, offset=SDp.offset,
                          ap=[[FSD, 125], [1, 10], [10, 2]])     # (bck, d in {0,1})
        sd_inA = bass.AP(tensor=psum_sd.tensor, offset=psum_sd.offset,
                         ap=[[FPS, 125], [3, 10], [1, 2]])
        nc.vector.tensor_copy(sd_outA, sd_inA)
        sd_outB = bass.AP(tensor=SDp.tensor, offset=SDp.offset + 20,
                          ap=[[FSD, 125], [1, 10]])              # d2 plane
        sd_inB = bass.AP(tensor=psum_sd.tensor, offset=psum_sd.offset + 2,
                         ap=[[FPS, 125], [3, 10]])
        nc.vector.tensor_copy(sd_outB, sd_inB)

        # --- stacked lhsT tiles; cols ordered (ts, b, ck) / (js, b, jq) ---
        SPK = data.tile([KS, 50], FP)
        nc.vector.memset(SPK[:], 0.0)
        TPK = data.tile([KT, 50], FP)
        nc.vector.memset(TPK[:], 0.0)
        onesrow = consts.tile([1, 50], FP)
        nc.vector.memset(onesrow[:], 1.0)
        nc.gpsimd.dma_start(TPK[N:N + 1, :], onesrow[:])
        FSK = SPK[:].ap[0][0]
        FTK = TPK[:].ap[0][0]

        def spat_rr(d, rbase, eng):
            for ts in range(5):
                src = bass.AP(tensor=SDp.tensor,
                              offset=SDp.offset + (ts * 25) * FSD + d * 10,
                              ap=[[FSD, N], [1, 10]])
                dst = bass.AP(tensor=SPK.tensor,
                              offset=SPK.offset + rbase * FSK + ts * 10,
                              ap=[[FSK, N], [1, 10]])
                eng.dma_start(dst, src)

        spat_rr(0, 0, nc.sync)
        spat_rr(1, N, nc.gpsimd)
        for js in range(5):
            for d, rbase in ((0, 0), (1, 32)):
                src = bass.AP(tensor=TDp.tensor,
                              offset=TDp.offset + (js * 25) * FTD + d * 10,
                              ap=[[FTD, N], [1, 10]])
                dst = bass.AP(tensor=TPK.tensor,
                              offset=TPK.offset + rbase * FTK + js * 10,
                              ap=[[FTK, N], [1, 10]])
                eng = nc.sync if d == 0 else nc.gpsimd
                eng.dma_start(dst, src)
        spat_rr(2, 64, nc.scalar)

        CSHt = consts.tile([114, NN], FP)
        csh_b = bass.AP(tensor=csh_d.tensor, offset=csh_d.offset, ap=[[0, 50], [1, NN]])
        nc.gpsimd.dma_start(CSHt[64:114, :], csh_b)

        # --- EC = exp(-sqrt(d2s)/1000) via exp(0.5*ln) on SPK rows 64:89 ---
        eps_b = consts.tile([89, 1], FP)
        nc.vector.memset(eps_b[:], 1e-30)
        ecL = data.tile([89, 50], FP)
        nc.scalar.activation(ecL[64:89, :], SPK[64:89, 0:50], AF.Ln,
                             bias=eps_b[64:89])
        ecW = data.tile([89, 50], FP)
        nc.scalar.activation(ecW[64:89, :], ecL[64:89, :], AF.Exp, scale=0.5)
        nc.scalar.activation(SPK[64:89, 0:50], ecW[64:89, :], AF.Exp, scale=-0.001)

        # --- E matmuls (stacked-K): spatial rows 0:50, temporal 64:114 ---
        nc.vector.memset(psum_E[32:64, 0:NN], 0.0)  # junk rows 50:64
        chunks = [(0, 512), (512, NN)]
        for lo, hi in chunks:
            nc.tensor.matmul(psum_E[64:114, lo:hi], TPK[:, :], qst[:, lo:hi],
                             start=True, stop=True, tile_position=(0, 64))
        for lo, hi in chunks:
            nc.tensor.matmul(psum_E[0:50, lo:hi], SPK[:, :], qss[:, lo:hi],
                             start=True, stop=True)

        # --- softmax tail (m is n2-major: groups are contiguous 25-runs) ---
        t0 = data.tile([114, NN], FP)
        E2 = data.tile([114, NN], FP)
        g = data.tile([114, NN], FP)
        Z = data.tile([114, N], FP)
        Zr = data.tile([114, N], FP)
        att1 = data.tile([114, NN], FP)
        g2 = data.tile([114, NN], FP)
        Z2 = data.tile([114, N], FP)
        Z2r = data.tile([114, N], FP)
        outF = data.tile([114, NN], FP)

        def gview(t, lo, hi, npart=114, p0=0):
            """[(p), (n2 groups), (n1 contiguous)] view."""
            fs = t[:].ap[0][0]
            return bass.AP(tensor=t.tensor, offset=t.offset + p0 * fs + lo * N,
                           ap=[[fs, npart], [N, hi - lo], [1, N]])

        def bview(t, lo, hi, npart=114, p0=0):
            """broadcast [(p), (n2), (n1 step-0)] view of a [*, 25] tile."""
            fs = t[:].ap[0][0]
            return bass.AP(tensor=t.tensor, offset=t.offset + p0 * fs + lo,
                           ap=[[fs, npart], [1, hi - lo], [0, N]])

        for lo, hi in N2SPLITS:
            cl, ch = lo * N, hi * N
            # LRelu: E2 = max(E, 0.2E); temporal rows get -csh
            nc.scalar.mul(t0[:, cl:ch], psum_E[:, cl:ch], 0.2)
            nc.vector.tensor_tensor(E2[:, cl:ch], psum_E[:, cl:ch], t0[:, cl:ch],
                                    op=ALU.max)
            nc.gpsimd.tensor_tensor(E2[64:114, cl:ch], E2[64:114, cl:ch],
                                    CSHt[64:114, cl:ch], op=ALU.subtract)
            # softmax 1
            nc.scalar.activation(g[:, cl:ch], E2[:, cl:ch], AF.Exp)
            nc.vector.tensor_reduce(Z[:, lo:hi], gview(g, lo, hi),
                                    axis=mybir.AxisListType.X, op=ALU.add)
            nc.vector.reciprocal(Zr[:, lo:hi], Z[:, lo:hi])
            nc.gpsimd.tensor_tensor(gview(att1, lo, hi), gview(g, lo, hi),
                                    bview(Zr, lo, hi), op=ALU.mult)
            # softmax 2
            nc.scalar.activation(g2[:, cl:ch], att1[:, cl:ch], AF.Exp)
            nc.vector.tensor_reduce(Z2[:, lo:hi], gview(g2, lo, hi),
                                    axis=mybir.AxisListType.X, op=ALU.add)
            nc.vector.reciprocal(Z2r[:, lo:hi], Z2[:, lo:hi])
            # final scale, writing transposed back to n1-major for output
            oswap = bass.AP(tensor=outF.tensor, offset=outF.offset + lo,
                            ap=[[outF[:].ap[0][0], 114], [1, hi - lo], [N, N]])
            nc.gpsimd.tensor_tensor(oswap, gview(g2, lo, hi),
                                    bview(Z2r, lo, hi), op=ALU.mult)

        # --- outputs: unscramble (ts,b,ck)-row order per ts / js ---
        FO = outF[:].ap[0][0]
        for ts in range(5):
            src = bass.AP(tensor=outF.tensor, offset=outF.offset + (ts * 10) * FO,
                          ap=[[FO, 10], [1, NN]])                # rows (b, ck)
            dst = bass.AP(tensor=outs_d.tensor, offset=outs_d.offset + ts * NN,
                          ap=[[25 * NN, BL], [5 * NN, 5], [1, NN]])  # (b, ck, m)
            (nc.sync if ts < 3 else nc.gpsimd).dma_start(dst, src)
        for js in range(5):
            src = bass.AP(tensor=outF.tensor, offset=outF.offset + (64 + js * 10) * FO,
                          ap=[[FO, 10], [1, NN]])                # rows (b, jq)
            dst = bass.AP(tensor=outt_d.tensor, offset=outt_d.offset + js * NN,
                          ap=[[25 * NN, BL], [5 * NN, 5], [1, NN]])  # (b, jq, m)
            (nc.scalar if js < 3 else nc.gpsimd).dma_start(dst, src)

    nc.compile()
    return nc


_PROGRAM = None


def _get_program():
    global _PROGRAM
    if _PROGRAM is None:
        _PROGRAM = _build_program()
    return _PROGRAM


# ------------------------------------------------------------------ kernel --

def kernel(src, W_s, a_s, W_t, a_t):
    from concourse.bass_utils import run_bass_kernel_spmd

    src = np.ascontiguousarray(np.asarray(src, dtype=np.float32))
    wa4, qstk_s, qstk_t, csh = _host_consts(np.asarray(W_s), np.asarray(a_s),
                                            np.asarray(W_t), np.asarray(a_t))
    nc = _get_program()
    in_maps = []
    for c in range(NCORES):
        in_maps.append({
            "src_l": src[c * BL:(c + 1) * BL],
            "wa4": wa4, "qstk_s": qstk_s, "qstk_t": qstk_t, "csh": csh,
        })
    res = run_bass_kernel_spmd(nc, in_maps, core_ids=list(range(NCORES)))
    out_s = np.concatenate([res.results[c]["out_s"] for c in range(NCORES)], axis=0)
    out_t = np.concatenate([res.results[c]["out_t"] for c in range(NCORES)], axis=0)
    return out_s, out_t



# revision 18
# speedup vs baseline: 1.4016x; 1.4016x over previous
"""Trainium2 Bass kernel for nn_DMS_STGAT (dual-branch GAT attention softmaxes).

Strategy (per core, data-parallel over batch B=16 -> 2 per core):
  The reference reduces to per-column dot products s1/s2/t1/t2 (128-dim,
  against host-precomputed Wa vectors) plus a scrambled-pair gather
  e[m] = LRelu(d1[r1[m]] + d2[r2[m]] (+ const)) and a double softmax
  over n1-groups.

  v2 design (vs the 49µs baseline):
  - dots: tiny wa stationary [128,2] STREAMS X (fp16, 1 cyc/row) instead of
    125-wide stationaries streaming 2 cols; three dot groups (spatial,
    temporal, d2-distance) run on col-groups 0/32/64 of the PE array
    concurrently.
  - the dot matmul rhs uses reordered 4D APs (j-major for spatial+d2,
    t-major for temporal) so every corner-turn DMA is a clean
    [25 part x 100B] block copy (3 DMAs total).
  - both branches share ONE stacked-K fp16 E-matmul [77,100]@[77,625]:
    rows 0:25 Q1, 25:50 Q2, 50:75 qs (spatial pe term), 75:76 qp hi/lo
    (temporal positional constant split into fp16 hi+lo ones-rows).
  - LeakyRelu in one DVE scalar_tensor_tensor op; exp-overflow safety via
    per-n2-group host constant csh subtracted post-LRelu (cancels in
    softmax).
"""
import sys
import numpy as np

for _p in ("/opt/trn_rl_repo", "/root/.axon_site/_ro/trn_rl_repo"):
    if _p not in sys.path:
        sys.path.insert(0, _p)

from contextlib import ExitStack  # noqa: E402

import concourse.bass as bass  # noqa: E402
import concourse.tile as tile  # noqa: E402
from concourse import bacc, mybir  # noqa: E402

B, C, T, J, F = 16, 128, 25, 25, 256
N = 25            # N == T == J
NN = N * N        # 625
NCORES = 8
BL = B // NCORES  # 2 batches per core
NC2 = BL * NN     # 1250 columns per core
FP = mybir.dt.float32
HF = mybir.dt.float16
AF = mybir.ActivationFunctionType
ALU = mybir.AluOpType

KE = 98           # E-matmul K rows: Q1@0:25, Q2@32:57, qs@64:89, qp@96:98
ME = 114          # E-matmul M cols: spatial 0:50, zero 50:64, temporal 64:114

# j/t chunking of the dot matmuls (psum bank = 512 fp32 per partition)
CHUNKS = [(0, 10), (10, 20), (20, 25)]
# m' chunking of the E matmul / softmax tail (groups of 25, n2-major)
MCHUNKS = [(0, 13, 0), (13, 25, 512)]  # (group lo, group hi, psum free off)

# Pin ALL activation functions to one table set (exp/ln/copy/prelu live
# together in natural_log_exp_and_others) so only one ACT_TABLE_LOAD happens.
_orig_get_tables = bacc.get_activation_tables


def _pinned_tables(arch):
    tabs = dict(_orig_get_tables(arch))
    assert "natural_log_exp_and_others" in tabs
    return {k: (v if k == "natural_log_exp_and_others" else set())
            for k, v in tabs.items()}


bacc.get_activation_tables = _pinned_tables

# ---------------------------------------------------------------- host math --

def _pair_indices():
    r1 = np.zeros(NN, np.int64)
    r2 = np.zeros(NN, np.int64)
    for m in range(NN):
        k1, k2 = 2 * m, 2 * m + 1
        r1[m] = (k1 // N) if k1 < NN else ((k1 - NN) % N)
        r2[m] = (k2 // N) if k2 < NN else ((k2 - NN) % N)
    return r1, r2


def _sinusoid_pos():
    pos = np.arange(200)[:, None].astype(np.float64)
    hid = np.arange(C)[None, :]
    angle = pos / np.power(10000.0, 2.0 * (hid // 2) / C)
    tab = angle.copy()
    tab[:, 0::2] = np.sin(angle[:, 0::2])
    tab[:, 1::2] = np.cos(angle[:, 1::2])
    return tab[:T] * 1000.0  # [T, C] float64


_R1, _R2 = _pair_indices()
# m' = n2*25 + n1  (n2-major so softmax n1-groups are contiguous runs)
_MPERM = (np.arange(NN) % N) * N + (np.arange(NN) // N)


def _host_consts(W_s, a_s, W_t, a_t):
    """Precompute tiny derived params in float64. ~0.3 MFLOP."""
    W_s = W_s.astype(np.float64)
    a_s = a_s.astype(np.float64)
    W_t = W_t.astype(np.float64)
    a_t = a_t.astype(np.float64)
    wa_s1 = W_s @ a_s[:F, 0]
    wa_s2 = W_s @ a_s[F:, 0]
    wa_t1 = W_t @ a_t[:F, 0]
    wa_t2 = W_t @ a_t[F:, 0]
    S1, S2 = wa_s1.sum(), wa_s2.sum()

    Q1 = np.zeros((N, NN), np.float64)
    Q2 = np.zeros((N, NN), np.float64)
    Q1[_R1, np.arange(NN)] = 1.0
    Q2[_R2, np.arange(NN)] = 1.0
    qs = S1 * Q1 + S2 * Q2

    pos = _sinusoid_pos()
    p1 = pos @ wa_t1
    p2 = pos @ wa_t2
    qp = (p1[_R1] + p2[_R2])[_MPERM]               # [625] m'-basis
    qp_hi = np.float16(qp).astype(np.float64)
    qp_lo = qp - qp_hi
    qLR = np.where(qp > 0, qp, 0.2 * qp)
    cq = qLR.reshape(N, N).max(axis=1)             # max over n1 per n2 group
    csh = np.repeat(cq, N)                         # [625] m'-basis

    qstk = np.zeros((KE, NN), np.float64)
    qstk[0:N] = Q1[:, _MPERM]
    qstk[32:32 + N] = Q2[:, _MPERM]
    qstk[64:64 + N] = qs[:, _MPERM]
    qstk[96] = qp_hi
    qstk[97] = qp_lo

    w6 = np.zeros((C, 5), np.float64)
    w6[:, 0] = wa_s1
    w6[:, 1] = wa_s2
    w6[:, 2] = wa_t1
    w6[:, 3] = wa_t2
    w6[:, 4] = 1.0

    csh50 = np.tile(csh[None, :], (2 * N, 1))
    return (w6.astype(np.float16), qstk.astype(np.float16),
            csh50.astype(np.float32))


# ------------------------------------------------------------- bass program --

def _build_program():
    nc = bacc.Bacc("TRN2", target_bir_lowering=False, debug=False)

    src_d = nc.dram_tensor("src_l", [BL, C, T, J], FP, kind="ExternalInput").ap()
    w6_d = nc.dram_tensor("w6", [C, 5], HF, kind="ExternalInput").ap()
    qstk_d = nc.dram_tensor("qstk", [KE, NN], HF, kind="ExternalInput").ap()
    csh_d = nc.dram_tensor("csh50", [2 * N, NN], FP, kind="ExternalInput").ap()
    outs_d = nc.dram_tensor("out_s", [BL, T, N, N], FP, kind="ExternalOutput").ap()
    outt_d = nc.dram_tensor("out_t", [BL, T, N, N], FP, kind="ExternalOutput").ap()

    with tile.TileContext(nc) as tc, ExitStack() as ctx:
        consts = ctx.enter_context(tc.tile_pool(name="consts", bufs=1))
        data = ctx.enter_context(tc.tile_pool(name="data", bufs=1))
        pp = ctx.enter_context(tc.tile_pool(name="pp", bufs=1, space="PSUM"))

        X = data.tile([C, NC2], FP)
        X16 = data.tile([C, NC2], HF)
        SUB = data.tile([C, NC2], HF)
        D2 = data.tile([C, NC2], HF)
        W6 = consts.tile([C, 5], HF)
        QS = consts.tile([KE, NN], HF)
        CSH = consts.tile([ME, NN], FP)  # csh50 lives in rows 64:114
        DOT = data.tile([66, NC2], HF)
        LK = data.tile([KE, ME], HF)
        eL = data.tile([89, 50], FP)   # rows 64:89 used (lane-locked with LK)
        eW = data.tile([89, 50], FP)
        eps_b = consts.tile([89, 1], FP)
        E2 = data.tile([ME, NN], FP)
        g = data.tile([ME, NN], FP)
        att1 = data.tile([ME, NN], FP)
        g2 = data.tile([ME, NN], FP)
        Z = data.tile([ME, N], FP)
        Zr = data.tile([ME, N], FP)
        Z2 = data.tile([ME, N], FP)
        Z2r = data.tile([ME, N], FP)
        outF = data.tile([ME, NN], FP)
        dummy = consts.tile([1, 2], FP)

        pd = [pp.tile([66, 512], FP, name=f"pd{k}") for k in range(3)]
        pe_ = pp.tile([ME, 1024], FP)

        FX = X[:].ap[0][0]
        FX16 = X16[:].ap[0][0]
        FSB = SUB[:].ap[0][0]
        FD2 = D2[:].ap[0][0]
        FD = DOT[:].ap[0][0]
        FL = LK[:].ap[0][0]

        # --- input DMAs first (transfers overlap the ACT table load) ---
        for b, eng in ((0, nc.sync), (1, nc.scalar)):
            src_b = bass.AP(tensor=src_d.tensor, offset=src_d.offset + b * C * NN,
                            ap=[[NN, C], [1, NN]])
            eng.dma_start(X[:, b * NN:(b + 1) * NN], src_b)
        nc.sync.dma_start(W6[:], w6_d)
        nc.scalar.dma_start(QS[:], qstk_d)
        nc.gpsimd.dma_start(CSH[64:ME, :], csh_d)

        # --- LK zeros + qp ones-rows; ACT table warm-up ---
        nc.vector.memset(LK[:], 0.0)
        nc.vector.memset(LK[96:98, 64:ME], 1.0)
        nc.vector.memset(dummy[:], 0.0)
        nc.scalar.activation(dummy[:], dummy[:], AF.Exp)

        # --- cast X -> fp16 (split across engines) ---
        nc.vector.tensor_copy(X16[:, 0:550], X[:, 0:550])
        nc.scalar.copy(X16[:, 550:1050], X[:, 550:1050])
        nc.gpsimd.tensor_copy(X16[:, 1050:1250], X[:, 1050:1250])

        # --- SUB = X16 - X16[j=8]; D2 = SUB^2 (per j-chunk, DVE) ---
        for j0, j1 in CHUNKS:
            jw = j1 - j0
            sview = bass.AP(tensor=SUB.tensor, offset=SUB.offset + j0,
                            ap=[[FSB, C], [NN, BL], [N, N], [1, jw]])
            xview = bass.AP(tensor=X16.tensor, offset=X16.offset + j0,
                            ap=[[FX16, C], [NN, BL], [N, N], [1, jw]])
            rview = bass.AP(tensor=X16.tensor, offset=X16.offset + 8,
                            ap=[[FX16, C], [NN, BL], [N, N], [0, jw]])
            dview = bass.AP(tensor=D2.tensor, offset=D2.offset + j0,
                            ap=[[FD2, C], [NN, BL], [N, N], [1, jw]])
            nc.vector.tensor_tensor(sview, xview, rview, op=ALU.subtract)
            nc.vector.tensor_tensor(dview, sview, sview, op=ALU.mult)

        # --- dot matmuls: wa stationary, X streams; col-groups 0/32/64 ---
        for k, (c0, c1) in enumerate(CHUNKS):
            cw = c1 - c0
            w = cw * 50
            rhsA = bass.AP(tensor=X16.tensor, offset=X16.offset + c0,
                           ap=[[FX16, C], [1, cw], [NN, BL], [N, N]])
            rhsB = bass.AP(tensor=X16.tensor, offset=X16.offset + c0 * N,
                           ap=[[FX16, C], [N, cw], [NN, BL], [1, N]])
            rhsC = bass.AP(tensor=D2.tensor, offset=D2.offset + c0,
                           ap=[[FD2, C], [1, cw], [NN, BL], [N, N]])
            nc.tensor.matmul(pd[k][0:2, 0:w], W6[:, 0:2], rhsA,
                             start=True, stop=True)
            nc.tensor.matmul(pd[k][32:34, 0:w], W6[:, 2:4], rhsB,
                             start=True, stop=True)
            nc.tensor.matmul(pd[k][64:65, 0:w], W6[:, 4:5], rhsC,
                             start=True, stop=True)

        # --- PSUM -> SBUF (cast to fp16) per chunk ---
        for k, (c0, c1) in enumerate(CHUNKS):
            w = (c1 - c0) * 50
            eng = (nc.vector, nc.scalar, nc.vector)[k]
            if eng is nc.scalar:
                eng.copy(DOT[0:66, c0 * 50:c0 * 50 + w], pd[k][0:66, 0:w])
            else:
                eng.tensor_copy(DOT[0:66, c0 * 50:c0 * 50 + w], pd[k][0:66, 0:w])

        # --- corner-turn DMAs: [25 part x 100B] block copies per plane ---
        def corner(eng, src_row, dst_poff, dst_coff):
            eng.dma_start(
                bass.AP(tensor=LK.tensor,
                        offset=LK.offset + dst_poff * FL + dst_coff,
                        ap=[[FL, N], [1, 50]]),
                bass.AP(tensor=DOT.tensor, offset=DOT.offset + src_row * FD,
                        ap=[[FD, 1], [50, N], [1, 50]]))

        corner(nc.sync, 0, 0, 0)       # s1 -> LK[0:25, 0:50]
        corner(nc.sync, 1, 32, 0)      # s2 -> LK[32:57, 0:50]
        corner(nc.gpsimd, 32, 0, 64)   # t1 -> LK[0:25, 64:114]
        corner(nc.gpsimd, 33, 32, 64)  # t2 -> LK[32:57, 64:114]
        corner(nc.scalar, 64, 64, 0)   # d2 -> LK[64:89, 0:50]

        # --- EC = exp(-sqrt(d2)/1000) in-place on LK[64:89, 0:50] ---
        nc.vector.memset(eps_b[:], 6.1035e-05)
        nc.scalar.activation(eL[64:89, :], LK[64:89, 0:50], AF.Ln,
                             bias=eps_b[64:89])
        nc.scalar.activation(eW[64:89, :], eL[64:89, :], AF.Exp, scale=0.5)
        nc.scalar.activation(LK[64:89, 0:50], eW[64:89, :], AF.Exp, scale=-0.001)

        # --- stacked E matmul: [98,114] @ [98,625] -> psum [114, 625] ---
        for glo, ghi, poff in MCHUNKS:
            w = (ghi - glo) * N
            nc.tensor.matmul(pe_[0:ME, poff:poff + w], LK[:, :],
                             QS[:, glo * N:glo * N + w], start=True, stop=True)

        # --- softmax tail (m' is n2-major: n1-groups are contiguous 25s) ---
        def gview(t, glo, ghi):
            fs = t[:].ap[0][0]
            return bass.AP(tensor=t.tensor, offset=t.offset + glo * N,
                           ap=[[fs, ME], [N, ghi - glo], [1, N]])

        def bview(t, glo, ghi):
            fs = t[:].ap[0][0]
            return bass.AP(tensor=t.tensor, offset=t.offset + glo,
                           ap=[[fs, ME], [1, ghi - glo], [0, N]])

        FO = outF[:].ap[0][0]
        for ci, (glo, ghi, poff) in enumerate(MCHUNKS):
            w = (ghi - glo) * N
            lo = glo * N
            pv = pe_[0:ME, poff:poff + w]
            # LeakyRelu in one ACT op (single PSUM input allowed)
            nc.scalar.activation(E2[:, lo:lo + w], pv, AF.Prelu, alpha=0.2)
            nc.gpsimd.tensor_tensor(E2[64:ME, lo:lo + w], E2[64:ME, lo:lo + w],
                                    CSH[64:ME, lo:lo + w], op=ALU.subtract)
            nc.scalar.activation(g[:, lo:lo + w], E2[:, lo:lo + w], AF.Exp)
            nc.vector.tensor_reduce(Z[:, glo:ghi], gview(g, glo, ghi),
                                    axis=mybir.AxisListType.X, op=ALU.add)
            nc.vector.reciprocal(Zr[:, glo:ghi], Z[:, glo:ghi])
            eng1 = nc.vector if ci == 0 else nc.gpsimd
            eng1.tensor_tensor(gview(att1, glo, ghi), gview(g, glo, ghi),
                               bview(Zr, glo, ghi), op=ALU.mult)
            nc.scalar.activation(g2[:, lo:lo + w], att1[:, lo:lo + w], AF.Exp)
            nc.vector.tensor_reduce(Z2[:, glo:ghi], gview(g2, glo, ghi),
                                    axis=mybir.AxisListType.X, op=ALU.add)
            nc.vector.reciprocal(Z2r[:, glo:ghi], Z2[:, glo:ghi])
            # final scale, writing transposed back to n1-major for output
            oswap = bass.AP(tensor=outF.tensor, offset=outF.offset + glo,
                            ap=[[FO, ME], [1, ghi - glo], [N, N]])
            eng2 = nc.gpsimd if ci == 0 else nc.vector
            eng2.tensor_tensor(oswap, gview(g2, glo, ghi),
                               bview(Z2r, glo, ghi), op=ALU.mult)

        # --- outputs: rows are (b, t)/(b, j) in natural order -> one DMA each
        nc.sync.dma_start(
            bass.AP(tensor=outs_d.tensor, offset=outs_d.offset,
                    ap=[[NN, 50], [1, NN]]),
            bass.AP(tensor=outF.tensor, offset=outF.offset, ap=[[FO, 50], [1, NN]]))
        nc.gpsimd.dma_start(
            bass.AP(tensor=outt_d.tensor, offset=outt_d.offset,
                    ap=[[NN, 50], [1, NN]]),
            bass.AP(tensor=outF.tensor, offset=outF.offset + 64 * FO,
                    ap=[[FO, 50], [1, NN]]))

    nc.compile()
    return nc


_PROGRAM = None


def _get_program():
    global _PROGRAM
    if _PROGRAM is None:
        _PROGRAM = _build_program()
    return _PROGRAM


# ------------------------------------------------------------------ kernel --

def kernel(src, W_s, a_s, W_t, a_t):
    from concourse.bass_utils import run_bass_kernel_spmd

    src = np.ascontiguousarray(np.asarray(src, dtype=np.float32))
    w6, qstk, csh50 = _host_consts(np.asarray(W_s), np.asarray(a_s),
                                   np.asarray(W_t), np.asarray(a_t))
    nc = _get_program()
    in_maps = []
    for c in range(NCORES):
        in_maps.append({
            "src_l": src[c * BL:(c + 1) * BL],
            "w6": w6, "qstk": qstk, "csh50": csh50,
        })
    res = run_bass_kernel_spmd(nc, in_maps, core_ids=list(range(NCORES)))
    out_s = np.concatenate([res.results[c]["out_s"] for c in range(NCORES)], axis=0)
    out_t = np.concatenate([res.results[c]["out_t"] for c in range(NCORES)], axis=0)
    return out_s, out_t
